# revision 65
# baseline (speedup 1.0000x reference)
"""NonLocalAttention2D Trainium2 kernel (v3).

Data-parallel over batch N=8: one image per NeuronCore.

Per-core math (x: (C=128, HW=4096) fp32):
  kv   = [Wv|Wk].T @ x              (80, 4096)  PE fp16 (v rows 0:64, k 64:80)
  pool = maxpool2x2(kv)+bias        (80, 1024)  ACT copy + DVE max -> kvf f32
  kb   = fp16(k rows -> part 0:16)  gpsimd cast DMA
  A_c  = Wq @ k_c -> ab fp16        (128, 1024) PE + DVE cast
  bqk  = k.T @ bq, ebqk = exp(bqk)  (128, 8)    PE + ACT
  vaugT= [vT*ebqk | ebqk]           (128, 8*65) PE transpose (f32) + DVE -> bf16
  s_cb = A_c.T @ x_b                (128k,512q) PE fp16 -> psum
  attn = exp(s)                     ACT -> bf16 sbuf  (paces the main loop)
  av   = vaugT.T @ attn  (accum 8c) (65, 512)   PE bf16; row 64 = denom
  r    = recip_approx_fast(denom)   (1, 512)    DVE (input staged to SBUF)
  R65  = broadcast r over 65 parts  DRAM-bounce DMA (PE ones-matmul on tail)
  aoTn = av * R65                   (65, 512)   DVE -> fp16 (row 64 == 1)
  fin  = [g*Wo; g*bo].T @ aoTn      (128, 512)  PE fp16
  out  = fin + x_b                  (128, 512)  DVE -> DMA out

Block 0's scores/exp are interleaved into the prologue so the ACT exp
stream (the pacing engine) starts as early as possible.
"""

import sys

if "/opt/trn_rl_repo" not in sys.path:
    sys.path.insert(0, "/opt/trn_rl_repo")

import numpy as np

import concourse.bacc as bacc
import concourse.bass as bass
import concourse.tile as tile
from concourse import bass_utils, mybir

F32 = mybir.dt.float32
F16 = mybir.dt.float16
BF16 = mybir.dt.bfloat16

C = 128          # channels
HW = 4096        # 64*64 pixels
L = 1024         # pooled keys (32*32)
D = 16           # attn dim
DV = 64          # value dim
KV = 80          # kv projection out width (v rows 0:64, k rows 64:80)
QB = 512         # q-block size
NB = HW // QB    # 8 q blocks
KC = 128         # keys per chunk
NCH = L // KC    # 8 key chunks
NCORES = 8
WBW = KV + C + C + 1 + DV  # w_kv | wqt | wfin | bq | ident64


def build_kernel():
    nc = bacc.Bacc("TRN2", target_bir_lowering=False, debug=False)

    x_d = nc.dram_tensor("x", (C, HW), F32, kind="ExternalInput").ap()
    wb_d = nc.dram_tensor("wb", (C, WBW), F16, kind="ExternalInput").ap()
    bkv_d = nc.dram_tensor("bkv", (C, 2), F32, kind="ExternalInput").ap()
    out_d = nc.dram_tensor("out", (C, HW), F32, kind="ExternalOutput").ap()

    from contextlib import ExitStack

    with tile.TileContext(nc) as tc, ExitStack() as ctx:
        singles = ctx.enter_context(tc.tile_pool(name="singles", bufs=1))
        s1_pool = ctx.enter_context(tc.tile_pool(name="s1", bufs=4))
        attn_pool = ctx.enter_context(tc.tile_pool(name="attn", bufs=2))
        r_pool = ctx.enter_context(tc.tile_pool(name="r", bufs=2))
        ao_pool = ctx.enter_context(tc.tile_pool(name="ao", bufs=2))
        out_pool = ctx.enter_context(tc.tile_pool(name="outp", bufs=3))
        dram_pool = ctx.enter_context(tc.tile_pool(name="dram", bufs=2, space="DRAM"))

        ps_sc = ctx.enter_context(tc.tile_pool(name="ps_sc", bufs=2, space="PSUM"))
        ps_av = ctx.enter_context(tc.tile_pool(name="ps_av", bufs=2, space="PSUM"))
        ps_fin = ctx.enter_context(tc.tile_pool(name="ps_fin", bufs=2, space="PSUM"))

        # ---- SBUF singles ----
        wb = singles.tile([C, WBW], F16, tag="wb")
        xf = singles.tile([C, HW], F32, tag="xf")
        xh = singles.tile([C, HW], F16, tag="xh")
        kvh = singles.tile([KV, L], F16, tag="kvh")  # v rows 0:64, k 64:80
        ab = singles.tile([C, L], F16, tag="ab")
        ones65 = singles.tile([1, DV + 1], BF16, tag="ones")

        w_kv = wb[:, 0:KV]
        w_qt = wb[0:D, KV : KV + C]
        w_fin = wb[0 : DV + 1, KV + C : KV + 2 * C]
        b_q = wb[0:D, KV + 2 * C : KV + 2 * C + 1]
        w_qt64 = wb[DV : DV + D, KV : KV + C]
        b_q64 = wb[DV : DV + D, KV + 2 * C : KV + 2 * C + 1]
        ici = KV + 2 * C + 1
        baux = singles.tile([C, 2], F32, tag="baux")
        wqbk = baux[:, 0:1]   # (128,1) Wq@bk
        bkbq = baux[:, 1:2]   # (128,1) all = bk.bq

        # ---- input DMAs: x pieces issued from different engines so the
        # transfers run in parallel across DMA queues ----
        nc.sync.dma_start(out=wb, in_=wb_d)
        nc.sync.dma_start(out=baux, in_=bkv_d)
        for g in range(8):
            sl = slice(g * QB, (g + 1) * QB)
            nc.sync.dma_start(out=xf[:, sl], in_=x_d[:, sl])

        nc.vector.memset(ones65, 1.0)
        identh = wb[0:DV, ici : ici + DV]

        def xh_cast(g):  # 512-col slices on ACT, piece-aligned
            sl = slice(g * QB, (g + 1) * QB)
            nc.scalar.copy(xh[:, sl], xf[:, sl])

        xh_cast(0)
        xh_cast(1)

        attn0 = attn_pool.tile([KC, NCH * QB], BF16, tag="attn")
        sc0 = [None] * 4

        def late_tail(c):
            # A_c matmul (PSUM via ps_fin ring), fp16 cast, block-0 scores
            csl = slice(c * KC, (c + 1) * KC)
            a_ps = ps_fin.tile([C, QB], F32, tag="fin", name=f"a{c}")
            nc.tensor.matmul(
                a_ps[:, 0:KC], lhsT=w_qt64, rhs=kvh[DV : DV + D, csl],
                start=True, stop=True, tile_position=(DV, 0),
            )
            nc.vector.tensor_scalar_add(ab[:, csl], a_ps[:, 0:KC], wqbk)
            t = c // 2
            if c % 2 == 0:
                sc0[t] = ps_sc.tile([KC, 2 * QB], F32, tag="sc", name=f"sc0_{t}")
            nc.tensor.matmul(
                sc0[t][:, (c % 2) * QB : (c % 2 + 1) * QB],
                lhsT=ab[:, csl],
                rhs=xh[:, 0:QB],
                start=True,
                stop=True,
            )
            # 512-wide exp per chunk: feeds ACT as soon as each chunk lands
            nc.scalar.activation(
                attn0[:, c * QB : (c + 1) * QB],
                sc0[t][:, (c % 2) * QB : (c % 2 + 1) * QB],
                mybir.ActivationFunctionType.Exp,
            )

        # ---- prologue: kv proj + pool chain, block-0 scores interleaved ----
        proj = None
        for c in range(NCH):
            j = c % 2
            if j == 0:
                proj = ps_sc.tile([KC, 2 * QB], F32, tag="sc", name=f"proj{c}")
            sl = slice(c * QB, (c + 1) * QB)
            nc.tensor.matmul(
                proj[:KV, j * QB : (j + 1) * QB],
                lhsT=w_kv,
                rhs=xh[:, sl],
                start=True,
                stop=True,
            )
            csl = slice(c * KC, (c + 1) * KC)
            # maxpool 2x2 via DVE pool_max: w-pairs then h-pairs
            pv = proj[:KV, j * QB : (j + 1) * QB].rearrange(
                "p (w two) -> p w two", two=2
            )
            if c + 2 < NCH:
                xh_cast(c + 2)  # stay two 512-pieces ahead of proj use
            s1 = s1_pool.tile([KV, 256], F32, tag="s1")
            nc.vector.tensor_copy(s1[:, :], pv[:, :, 0])
            nc.vector.tensor_max(s1[:, :], s1[:, :], pv[:, :, 1])
            sv = s1.rearrange("p (h two w) -> p h two w", h=4, two=2)
            nc.vector.tensor_max(kvh[:, csl], sv[:, :, 0, :], sv[:, :, 1, :])
            if c >= 1:
                late_tail(c - 1)
        late_tail(NCH - 1)

        ebqk = singles.tile([KC, NCH], F32, tag="ebqk")
        vaug = singles.tile([KC, NCH * (DV + 1)], BF16, tag="vaug")

        def defer_kv_aux():
            # bqk, ebqk, vT transposes, vaug assembly (needed before av(0))
            vt_t = ps_fin.tile([C, QB], F32, tag="fin")  # 8x(128,64) vT chunks
            vt16 = vt_t.bitcast(F16)
            bqk_t = ps_fin.tile([C, QB], F32, tag="fin")  # cols 0:8 used
            for c in range(NCH):
                csl = slice(c * KC, (c + 1) * KC)
                nc.tensor.matmul(
                    bqk_t[:, c : c + 1], lhsT=kvh[DV : DV + D, csl], rhs=b_q64,
                    start=True, stop=True, tile_position=(DV, 0),
                )
                nc.tensor.transpose(
                    vt16[:, c * DV : (c + 1) * DV], kvh[0:DV, csl], identh
                )
            nc.scalar.activation(
                ebqk[:, :], bqk_t[:, 0:NCH],
                mybir.ActivationFunctionType.Exp, bias=bkbq,
            )
            for c in range(NCH):
                base = c * (DV + 1)
                nc.vector.tensor_scalar_mul(
                    vaug[:, base : base + DV],
                    vt16[:, c * DV : (c + 1) * DV],
                    ebqk[:, c : c + 1],
                )
                nc.vector.tensor_copy(
                    vaug[:, base + DV : base + DV + 1], ebqk[:, c : c + 1]
                )

        # ---- main loop: 4-deep software pipeline (block 0 prefilled) ----
        # iter i: PE [sc(i) x8 | av(i-1) x8 | fin(i-3)]
        #         ACT [exp(i) x4]
        #         DVE [dn+recip(i-1), aoTn-mul(i-2), residual-add(i-3)]
        #         DMA [r bounce (i-2), out (i-3)]
        attn_t, av_t, r_t, R65s_t, ao_t = {}, {}, {}, {}, {}
        attn_t[0] = attn0

        for i in range(1, NB + 4):
            b_sc = i          # scores + exp
            b_av = i - 1      # av accumulation + recip
            b_r = i - 2       # broadcast + aoTn mul
            b_f = i - 3       # fin + residual + store

            if b_sc < NB:
                qsl = slice(b_sc * QB, (b_sc + 1) * QB)
                attn = attn_pool.tile([KC, NCH * QB], BF16, tag="attn")
                attn_t[b_sc] = attn
                for t in range(4):
                    sc = ps_sc.tile([KC, 2 * QB], F32, tag="sc")
                    for j in range(2):
                        cc = 2 * t + j
                        nc.tensor.matmul(
                            sc[:, j * QB : (j + 1) * QB],
                            lhsT=ab[:, cc * KC : (cc + 1) * KC],
                            rhs=xh[:, qsl],
                            start=True,
                            stop=True,
                        )
                    # interleave av MMs of previous block between score tiles
                    if t == 1:
                        if i == 1:
                            defer_kv_aux()
                        elif 0 <= b_av < NB:
                            _av_mms(nc, ps_av, av_t, vaug, attn_t, b_av, 0, 4)
                    if t == 3 and 0 <= b_av < NB:
                        c0 = 0 if i == 1 else 4
                        _av_mms(nc, ps_av, av_t, vaug, attn_t, b_av, c0, 8)
                    nc.scalar.activation(
                        attn[:, t * 2 * QB : (t + 1) * 2 * QB],
                        sc[:, :],
                        mybir.ActivationFunctionType.Exp,
                    )
                if b_sc == NB - 1:
                    # last block: start av(7) chunks 0-3 as soon as its first
                    # exps land (rest in the next iteration)
                    _av_mms(nc, ps_av, av_t, vaug, attn_t, b_sc, 0, 4)
            elif 0 <= b_av < NB:
                c0 = 4 if b_av == NB - 1 else 0
                _av_mms(nc, ps_av, av_t, vaug, attn_t, b_av, c0, 8)

            if 0 <= b_av < NB:
                # recip of denominators as soon as av(b_av) stops
                # (custom-DVE recip must read SBUF: stage the psum row first)
                dn = r_pool.tile([1, QB], F32, tag="dn", name=f"dn{b_av}")
                r = r_pool.tile([1, QB], F32, tag="r", name=f"r{b_av}")
                nh = 2 if b_av >= NB - 2 else 1
                for h in range(nh):
                    hs = slice(h * QB // nh, (h + 1) * QB // nh)
                    nc.vector.tensor_copy(dn[:, hs], av_t[b_av][DV : DV + 1, hs])
                    nc.vector.reciprocal_approx_fast(r[:, hs], dn[:, hs])
                r_t[b_av] = r

            if 0 <= b_r < NB:
                R65s = r_pool.tile([DV + 1, QB], F32, tag="R65s", name=f"R65s{b_r}")
                if b_r < NB - 2:
                    # broadcast r over 65 partitions via DRAM bounce (partition
                    # stride 0 on the read); hidden by the 4-deep pipeline
                    r_dram = dram_pool.tile([1, QB], F32, tag="rd", name=f"rd{b_r}")
                    nc.sync.dma_start(out=r_dram[:, :], in_=r_t[b_r][:, :])
                    r_bcast = bass.AP(
                        tensor=r_dram.tensor,
                        offset=r_dram.offset,
                        ap=[[0, DV + 1], [1, QB]],
                    )
                    nc.sync.dma_start(out=R65s[:, :], in_=r_bcast)
                else:
                    # tail blocks: low-latency PE ones-matmul broadcast,
                    # 256-wide halves to pipeline the serial DVE chain
                    rb = r_pool.tile([1, QB], BF16, tag="rb", name=f"rb{b_r}")
                    R65p = ps_fin.tile([C, QB], F32, tag="fin")
                    for h in range(2):
                        hs = slice(h * 256, (h + 1) * 256)
                        nc.vector.tensor_copy(rb[:, hs], r_t[b_r][:, hs])
                        nc.tensor.matmul(
                            R65p[0 : DV + 1, hs], lhsT=ones65, rhs=rb[:, hs],
                            start=True, stop=True,
                        )
                        nc.vector.tensor_copy(R65s[:, hs], R65p[0 : DV + 1, hs])
                R65s_t[b_r] = R65s
                ao = ao_pool.tile([DV + 1, QB], F16, tag="ao")
                ao_t[b_r] = ao
                if b_r >= NB - 2:
                    for h in range(2):
                        hs = slice(h * 256, (h + 1) * 256)
                        nc.vector.tensor_mul(
                            ao[:, hs], av_t[b_r][:, hs], R65s[:, hs]
                        )
                else:
                    nc.vector.tensor_mul(ao[:, :], av_t[b_r][:, :], R65s[:, :])

            if 0 <= b_f < NB:
                qsl = slice(b_f * QB, (b_f + 1) * QB)
                fin = ps_fin.tile([C, QB], F32, tag="fin")
                o = out_pool.tile([C, QB], F32, tag="o")
                if b_f >= NB - 2:
                    for h in range(2):
                        hs = slice(h * 256, (h + 1) * 256)
                        qh = slice(b_f * QB + h * 256, b_f * QB + (h + 1) * 256)
                        nc.tensor.matmul(
                            fin[:, hs], lhsT=w_fin, rhs=ao_t[b_f][:, hs],
                            start=True, stop=True,
                        )
                        nc.vector.tensor_add(o[:, hs], fin[:, hs], xf[:, qh])
                        nc.sync.dma_start(out=out_d[:, qh], in_=o[:, hs])
                else:
                    nc.tensor.matmul(
                        fin[:, :], lhsT=w_fin, rhs=ao_t[b_f][:, :],
                        start=True, stop=True,
                    )
                    nc.vector.tensor_add(o[:, :], fin[:, :], xf[:, qsl])
                    nc.sync.dma_start(out=out_d[:, qsl], in_=o[:, :])

    nc.compile()
    return nc


def _av_mms(nc, ps_av, av_t, vaug, attn_t, b, c0, c1):
    if b not in av_t:
        av_t[b] = ps_av.tile([DV + 1, QB], F32, tag="av", name=f"av{b}")
    av = av_t[b]
    attn = attn_t[b]
    for c in range(c0, c1):
        base = c * (DV + 1)
        nc.tensor.matmul(
            av[:, :],
            lhsT=vaug[:, base : base + DV + 1],
            rhs=attn[:, c * QB : (c + 1) * QB],
            start=(c == 0),
            stop=(c == NCH - 1),
        )


def prep_weights(Wq, bq, Wk, bk, Wv, bv, Wo, bo, gamma):
    g = np.float32(np.asarray(gamma))
    Wq, Wk, Wv, Wo = (np.asarray(a, np.float32) for a in (Wq, Wk, Wv, Wo))
    bq_, bk_, bv_, bo_ = (np.asarray(a, np.float32) for a in (bq, bk, bv, bo))
    wblob = np.zeros((C, WBW), np.float16)
    wblob[:, 0:DV] = Wv.astype(np.float16)           # v cols 0:64
    wblob[:, DV : DV + D] = Wk.astype(np.float16)    # k cols 64:80
    wblob[0:D, KV : KV + C] = Wq.T.astype(np.float16)
    wblob[DV : DV + D, KV : KV + C] = Wq.T.astype(np.float16)
    wblob[0:DV, KV + C : KV + 2 * C] = (g * Wo).astype(np.float16)
    # bo' = bo + Wo.T bv  (v-bias folded host-side)
    wblob[DV, KV + C : KV + 2 * C] = (g * (bo_ + Wo.T @ bv_)).astype(np.float16)
    wblob[0:D, KV + 2 * C] = bq_.astype(np.float16)
    wblob[DV : DV + D, KV + 2 * C] = bq_.astype(np.float16)
    ici = KV + 2 * C + 1
    wblob[0:DV, ici : ici + DV] = np.eye(DV, dtype=np.float16)
    bkv = np.zeros((C, 2), np.float32)
    bkv[:, 0] = Wq @ bk_                  # A bias col (Wq@bk)
    bkv[:, 1] = np.float32(bk_ @ bq_)     # bqk scalar bias
    return np.ascontiguousarray(wblob), np.ascontiguousarray(bkv)


_NC_CACHE = {}


def kernel(x, Wq, bq, Wk, bk, Wv, bv, Wo, bo, gamma):
    x = np.asarray(x, dtype=np.float32)
    N = x.shape[0]
    assert x.shape == (N, C, 64, 64) and N == NCORES
    wblob, bkv = prep_weights(Wq, bq, Wk, bk, Wv, bv, Wo, bo, gamma)

    if "nc" not in _NC_CACHE:
        _NC_CACHE["nc"] = build_kernel()
    nc = _NC_CACHE["nc"]

    in_maps = []
    for i in range(N):
        in_maps.append(
            {
                "x": np.ascontiguousarray(x[i].reshape(C, HW)),
                "wb": wblob,
                "bkv": bkv,
            }
        )
    res = bass_utils.run_bass_kernel_spmd(nc, in_maps, core_ids=list(range(N)))
    out = np.stack([res.results[i]["out"].reshape(C, 64, 64) for i in range(N)])
    return out.astype(np.float32)


if __name__ == "__main__":
    print("built", build_kernel())
